# revision 20
# baseline (speedup 1.0000x reference)
"""Trainium2 Bass kernel for a separable 2-D db4 DWT (pywt 'symmetric' mode).

Input  x: [8, 512, 512, 16] f32, dec_lo: [8] f32.
Output (aa, ad, da, dd): each [8, 259, 259, 16] f32.

Sharding: pure data parallel — batch i runs on core i (8 cores).

Per-core algorithm (x1: [512, 512, 16]): channels are split between two
pipelines so every engine contributes:

PE path (channels 0..15-F): both DWT stages are banded matmuls with the
*data* as the stationary operand, so the stage-1 result comes out transposed
(W on partitions) exactly as stage 2 needs it:
    stage 1:  out1[w, (s,ho)]      = sum_k Xp[k, w]   * B[k, (s,ho)]
    stage 2:  out2[(s,ho),(s',wo)] = sum_k T1[k,(s,ho)] * B[k,(s',wo)]
B is a [128, 120] banded filter matrix: B[k, s*60+j] = f_s_rev[k-(2j+1)],
covering 60 output positions of both subbands per K=128 input window.
This path is LDWEIGHTS-bound (fp32 stationary reload per matmul), so the
remaining F channels go to:

VE path (channels 16-F..15): stage 1 is the W-axis conv done as 16 fused
multiply-add sweeps (scalar_tensor_tensor) on Vector/GpSimd over the free
axis; stage 2 is the H-contraction on PE with the *band* stationary
(cheap, N~480 moving), landing (s,ho) on partitions.

Both paths interleave their stage-2 results into shared staging tiles so
output DMAs write full (wo, c) rows (3.8 KB contiguous runs).
Output tiling: position blocks of 60 (window stride 120), 5 blocks per axis.
Symmetric padding (7 each side) is materialized in SBUF: H pad via mirrored
row DMAs, W pad via small on-chip column copies.
"""

from contextlib import ExitStack

import numpy as np

B_, H_, W_, C_ = 8, 512, 512, 16
L, PAD = 8, 7
OUT = (H_ + L - 1) // 2            # 259
T = 60                             # output positions per tile
NT = (OUT + T - 1) // T            # 5
WSTRIDE = 2 * T                    # 120 (input window stride)
NPAD = H_ + 2 * PAD                # 526
N_CORES = 8
BANDW = 2 * T                      # 120 band columns (full tile)
CNT_LAST = OUT - T * (NT - 1)      # 19 positions in the last tile
BANDT = BANDW + 2 * CNT_LAST       # +38 packed columns for the last tile
F_VE = 8                           # channels on the VE path
MM_DTYPE = "float32"


def _tile_params(t):
    cnt = min(T, OUT - T * t)      # output positions in tile t
    k0 = WSTRIDE * t               # padded-axis window start
    kw = min(128, NPAD - k0)       # window size (contraction K)
    return cnt, k0, kw


def _filters(dec_lo):
    dec_lo = np.asarray(dec_lo, np.float32)
    signs = np.where(np.arange(L) % 2 == 0, -1.0, 1.0).astype(np.float32)
    dec_hi = signs * dec_lo[::-1]
    return dec_lo[::-1].copy(), dec_hi[::-1].copy()


def band_matrix(dec_lo):
    lo_rev, hi_rev = _filters(dec_lo)
    B = np.zeros((128, BANDT), np.float32)
    for s, f in enumerate((lo_rev, hi_rev)):
        for j in range(T):
            for m in range(L):
                k = 2 * j + 1 + m
                if k < 128:
                    B[k, s * T + j] = f[m]
        for j in range(CNT_LAST):  # packed last-tile band (s-major, tight)
            for m in range(L):
                k = 2 * j + 1 + m
                B[k, BANDW + s * CNT_LAST + j] = f[m]
    return B


def taps_matrix(dec_lo):
    lo_rev, hi_rev = _filters(dec_lo)
    return np.tile(np.concatenate([lo_rev, hi_rev])[None, :], (128, 1)).copy()


def build_nc(dec_lo):
    import concourse.bacc as bacc
    import concourse.mybir as mybir
    import concourse.tile as tile

    f32 = mybir.dt.float32
    mm_dt = getattr(mybir.dt, MM_DTYPE)

    def _mmcast(ap):
        return ap if mm_dt == f32 else ap.bitcast(mm_dt)

    lo_rev, hi_rev = _filters(dec_lo)
    tap_coefs = [[float(v) for v in f] for f in (lo_rev, hi_rev)]
    CPE = C_ - F_VE                 # channels on the PE path
    CV0 = CPE                       # first VE channel
    pe_groups = [list(range(g, min(g + 4, CPE))) for g in range(0, CPE, 4)]

    nc = bacc.Bacc("TRN2", debug=False, num_devices=N_CORES)
    x = nc.dram_tensor("x", [H_, W_, C_], f32, kind="ExternalInput").ap()
    band = nc.dram_tensor("band", [128, BANDT], f32, kind="ExternalInput").ap()

    out_names = {(0, 0): "aa", (0, 1): "ad", (1, 0): "da", (1, 1): "dd"}
    outs = {
        k: nc.dram_tensor(v, [OUT, OUT, C_], f32, kind="ExternalOutput").ap()
        for k, v in out_names.items()
    }

    with tile.TileContext(nc) as tc, ExitStack() as ctx:
        xp_pool = ctx.enter_context(tc.tile_pool(name="xp", bufs=2))
        const_pool = ctx.enter_context(tc.tile_pool(name="consts", bufs=1))
        t1_pool = ctx.enter_context(tc.tile_pool(name="t1", bufs=2))
        vout_pool = ctx.enter_context(tc.tile_pool(name="vout", bufs=2))
        vtmp_pool = ctx.enter_context(tc.tile_pool(name="vtmp", bufs=1))
        stg_pool = ctx.enter_context(tc.tile_pool(name="stg", bufs=3))
        ps1_pool = ctx.enter_context(tc.tile_pool(name="ps1", bufs=3, space="PSUM"))
        ps2_pool = ctx.enter_context(tc.tile_pool(name="ps2", bufs=3, space="PSUM"))
        psv_pool = ctx.enter_context(tc.tile_pool(name="psv", bufs=2, space="PSUM"))

        bt = const_pool.tile([128, BANDT], f32)
        nc.sync.dma_start(bt[:], band[:])

        warm_ps = ps2_pool.tile([128, BANDW], f32, tag="ps2")
        for _ in range(96):  # keep HAM warm during the first input DMA
            nc.tensor.matmul(
                warm_ps[0:BANDW, :], bt[:, 0:BANDW], bt[:, 0:BANDW],
                start=True, stop=True,
            )

        def band_ap(kw, cnt):
            # contiguous [kw, 2*cnt] band slice (packed alternate for last tile)
            if cnt == T:
                return bt[0:kw, 0:BANDW]
            return bt[0:kw, BANDW:BANDW + 2 * cnt]

        # VE-path MAC balance: DVE runs fused STT MACs; GpSimd taps are an
        # ACT scale-copy into scratch plus a GpSimd tensor_tensor add
        # (Pool has no TensorScalarPtr on TRN2).
        ve_load = {"v": 0.0, "g": 0.0}
        ve_rate = {"v": 123.0, "g": 77.0}

        def ve_pick(nelem):
            key = min(ve_load, key=lambda k: ve_load[k] + nelem / ve_rate[k])
            ve_load[key] += nelem / ve_rate[key]
            return key

        for t in range(NT):
            cnt, k0, kw = _tile_params(t)
            ncols = 2 * cnt
            xp = xp_pool.tile([128, NPAD * C_], f32, tag="xp")

            # ---- load H-window (rows k0..k0+kw of padded H) with W pad ----
            p_lo = PAD - k0 if k0 < PAD else 0          # first interior partition
            hx_lo = max(0, k0 - PAD)
            p_hi = min(kw, H_ + PAD - k0)               # one past last interior
            hx_hi = hx_lo + (p_hi - p_lo)
            for tp in range(NT):                        # body, per wp-window chunk
                _, w0, ww = _tile_params(tp)
                wa = max(PAD, w0)                       # first body wp of chunk
                wb = min(PAD + W_, w0 + ww)
                load_eng = nc.sync if tp % 2 == 0 else nc.gpsimd
                load_eng.dma_start(
                    xp[p_lo:p_hi, wa * C_:wb * C_],
                    x[hx_lo:hx_hi, wa - PAD:wb - PAD, :].rearrange(
                        "h w c -> h (w c)"
                    ),
                )
            for p in range(0, p_lo):                    # top H mirror (t == 0)
                nc.sync.dma_start(
                    xp[p:p + 1, PAD * C_:(PAD + W_) * C_],
                    x[6 - p - k0:7 - p - k0].rearrange("h w c -> h (w c)"),
                )
            for p in range(p_hi, kw):                   # bottom H mirror (last t)
                hx = 2 * H_ - 1 + PAD - k0 - p
                nc.sync.dma_start(
                    xp[p:p + 1, PAD * C_:(PAD + W_) * C_],
                    x[hx:hx + 1].rearrange("h w c -> h (w c)"),
                )
            for j in range(PAD):                        # W mirror columns
                nc.gpsimd.tensor_copy(
                    xp[0:kw, j * C_:(j + 1) * C_],
                    xp[0:kw, (2 * PAD - 1 - j) * C_:(2 * PAD - j) * C_],
                )
                dst = NPAD - PAD + j
                src = PAD + W_ - 1 - j
                nc.gpsimd.tensor_copy(
                    xp[0:kw, dst * C_:(dst + 1) * C_],
                    xp[0:kw, src * C_:(src + 1) * C_],
                )

            # ---- VE stage 1: W-axis conv -> vout[p, (s', wo, cV)] ----
            if F_VE:
                vout = vout_pool.tile([128, 2 * OUT * F_VE], f32, tag="vout")
                xp_w = xp[0:kw, :].rearrange("p (w c) -> p w c", c=C_)
                vo = vout[0:kw, :].rearrange(
                    "p (s w c) -> p s w c", s=2, w=OUT
                )
                DVE_TAPS = (1, 2, 3, 5, 7)   # fused STT MACs on DVE
                for sp in range(2):
                    dst = vo[:, sp, :, :]

                    def tap_src(m):
                        return xp_w[:, 1 + m:1 + m + 2 * OUT:2, CV0:C_]

                    # chain A (ACT init + DVE STT accumulation)
                    nc.scalar.mul(dst, tap_src(0), tap_coefs[sp][0])
                    for m in DVE_TAPS:
                        nc.vector.scalar_tensor_tensor(
                            dst, tap_src(m), tap_coefs[sp][m], dst,
                            mybir.AluOpType.mult, mybir.AluOpType.add,
                        )
                    # chain B (ACT scales + GpSimd adds), combined at the end
                    vacc = vtmp_pool.tile([128, OUT * F_VE], f32, tag="vacc")
                    vtmp = vtmp_pool.tile([128, OUT * F_VE], f32, tag="vtmp")
                    vacc_r = vacc[0:kw, :].rearrange("p (w c) -> p w c", c=F_VE)
                    vtmp_r = vtmp[0:kw, :].rearrange("p (w c) -> p w c", c=F_VE)
                    nc.scalar.mul(vacc_r, tap_src(4), tap_coefs[sp][4])
                    nc.scalar.mul(vtmp_r, tap_src(6), tap_coefs[sp][6])
                    nc.gpsimd.tensor_add(vacc_r, vacc_r, vtmp_r)
                    # combine on DVE: keeps the slow GpSimd op off the
                    # S2ve critical path
                    nc.vector.tensor_add(dst, dst, vacc_r)

            # ---- PE stage 1: contract H -> T1[wp-win][w, cI*120 + (s,ho)] ----
            t1 = []
            for tp in range(NT):
                _, w0, ww = _tile_params(tp)
                t1t = t1_pool.tile([128, CPE * WSTRIDE], f32, tag=f"t1_{tp}")
                for grp in pe_groups:
                    gs = len(grp)
                    ps = ps1_pool.tile([128, 4 * WSTRIDE], f32, tag="ps1")
                    for ci, c in enumerate(grp):
                        lhsT = xp[0:kw, :].rearrange("k (w c) -> k w c", c=C_)[
                            :, w0:w0 + ww, c
                        ]
                        nc.tensor.matmul(
                            ps[0:ww, ci * WSTRIDE:ci * WSTRIDE + ncols],
                            _mmcast(lhsT),
                            _mmcast(band_ap(kw, cnt)),
                            start=True,
                            stop=True,
                        )
                    nc.scalar.copy(
                        t1t[0:ww, grp[0] * WSTRIDE:grp[0] * WSTRIDE + gs * WSTRIDE]
                        .rearrange("p (ci z) -> p ci z", ci=gs)[:, :, 0:ncols],
                        ps[0:ww, 0:gs * WSTRIDE]
                        .rearrange("p (ci z) -> p ci z", ci=gs)[:, :, 0:ncols],
                    )
                t1.append(t1t)

            # ---- stage 2 per wo-block: PE path + VE path -> shared stg ----
            for t2 in range(NT):
                cnt2, _, kw2 = _tile_params(t2)
                ncols2 = 2 * cnt2
                stg = stg_pool.tile([128, 2 * T * C_], f32, tag="stg")
                stg_r = stg[0:ncols, :].rearrange(
                    "p (s j c) -> p s j c", s=2, j=T
                )
                for grp in pe_groups:
                    gs = len(grp)
                    ps2 = ps2_pool.tile([128, 4 * WSTRIDE], f32, tag="ps2")
                    for ci, c in enumerate(grp):
                        lhsT = t1[t2][0:kw2, c * WSTRIDE:c * WSTRIDE + ncols]
                        nc.tensor.matmul(
                            ps2[0:ncols, ci * WSTRIDE:ci * WSTRIDE + ncols2],
                            _mmcast(lhsT),
                            _mmcast(band_ap(kw2, cnt2)),
                            start=True,
                            stop=True,
                        )
                    src = (
                        ps2[0:ncols, 0:gs * WSTRIDE]
                        .rearrange("p (ci z) -> p ci z", ci=gs)[:, :, 0:ncols2]
                        .rearrange("p ci (s j) -> p ci s j", s=2)
                    )
                    dst = (
                        stg_r[:, :, 0:cnt2, grp[0]:grp[0] + gs]
                        .transpose([0, 3, 1, 2])
                    )
                    nc.vector.tensor_copy(dst, src)
                # VE path stage 2: band-stationary matmul over H window
                if F_VE:
                    for sp in range(2):
                        psv = psv_pool.tile([128, T * F_VE], f32, tag="psv")
                        rhs = vo[:, sp, t2 * T:t2 * T + cnt2, :]
                        nc.tensor.matmul(
                            psv[0:ncols, 0:cnt2 * F_VE],
                            _mmcast(band_ap(kw, cnt)),
                            _mmcast(rhs),
                            start=True,
                            stop=True,
                        )
                        nc.scalar.copy(
                            stg_r[:, sp, 0:cnt2, CV0:C_],
                            psv[0:ncols, 0:cnt2 * F_VE]
                            .rearrange("p (j c) -> p j c", c=F_VE),
                        )
                for s in range(2):
                    for sp in range(2):
                        store_eng = nc.gpsimd if (s, sp) == (1, 0) else nc.scalar
                        store_eng.dma_start(
                            outs[(s, sp)][
                                t * T:t * T + cnt, t2 * T:t2 * T + cnt2, :
                            ].rearrange("h w c -> h (w c)"),
                            stg[s * cnt:(s + 1) * cnt,
                                sp * T * C_:sp * T * C_ + cnt2 * C_],
                        )

    nc.compile()
    return nc


_NC = {}


def _get_nc(dec_lo):
    key = np.asarray(dec_lo, np.float32).tobytes()
    if key not in _NC:
        _NC[key] = build_nc(dec_lo)
    return _NC[key]


def kernel(x, dec_lo):
    from concourse import bass_utils

    x = np.ascontiguousarray(np.asarray(x, np.float32))
    band = band_matrix(dec_lo)
    nc = _get_nc(dec_lo)
    in_maps = [{"x": x[i], "band": band} for i in range(N_CORES)]
    res = bass_utils.run_bass_kernel_spmd(nc, in_maps, core_ids=list(range(N_CORES)))
    names = ["aa", "ad", "da", "dd"]
    return tuple(
        np.stack([res.results[i][n] for i in range(N_CORES)], axis=0) for n in names
    )


# revision 21
# speedup vs baseline: 1.0305x; 1.0305x over previous
"""Trainium2 Bass kernel for a separable 2-D db4 DWT (pywt 'symmetric' mode).

Input  x: [8, 512, 512, 16] f32, dec_lo: [8] f32.
Output (aa, ad, da, dd): each [8, 259, 259, 16] f32.

Sharding: pure data parallel — batch i runs on core i (8 cores).

Per-core algorithm (x1: [512, 512, 16]): channels are split between two
pipelines so every engine contributes:

PE path (channels 0..15-F): both DWT stages are banded matmuls with the
*data* as the stationary operand, so the stage-1 result comes out transposed
(W on partitions) exactly as stage 2 needs it:
    stage 1:  out1[w, (s,ho)]      = sum_k Xp[k, w]   * B[k, (s,ho)]
    stage 2:  out2[(s,ho),(s',wo)] = sum_k T1[k,(s,ho)] * B[k,(s',wo)]
B is a [128, 120] banded filter matrix: B[k, s*60+j] = f_s_rev[k-(2j+1)],
covering 60 output positions of both subbands per K=128 input window.
This path is LDWEIGHTS-bound (fp32 stationary reload per matmul), so the
remaining F channels go to:

VE path (channels 16-F..15): stage 1 is the W-axis conv done as 16 fused
multiply-add sweeps (scalar_tensor_tensor) on Vector/GpSimd over the free
axis; stage 2 is the H-contraction on PE with the *band* stationary
(cheap, N~480 moving), landing (s,ho) on partitions.

Both paths interleave their stage-2 results into shared staging tiles so
output DMAs write full (wo, c) rows (3.8 KB contiguous runs).
Output tiling: position blocks of 60 (window stride 120), 5 blocks per axis.
Symmetric padding (7 each side) is materialized in SBUF: H pad via mirrored
row DMAs, W pad via small on-chip column copies.
"""

from contextlib import ExitStack

import numpy as np

B_, H_, W_, C_ = 8, 512, 512, 16
L, PAD = 8, 7
OUT = (H_ + L - 1) // 2            # 259
T = 60                             # output positions per tile
NT = (OUT + T - 1) // T            # 5
WSTRIDE = 2 * T                    # 120 (input window stride)
NPAD = H_ + 2 * PAD                # 526
N_CORES = 8
BANDW = 2 * T                      # 120 band columns (full tile)
CNT_LAST = OUT - T * (NT - 1)      # 19 positions in the last tile
BANDT = BANDW + 2 * CNT_LAST       # +38 packed columns for the last tile
F_VE = 6                           # channels on the VE path
MM_DTYPE = "float32"


def _tile_params(t):
    cnt = min(T, OUT - T * t)      # output positions in tile t
    k0 = WSTRIDE * t               # padded-axis window start
    kw = min(128, NPAD - k0)       # window size (contraction K)
    return cnt, k0, kw


def _filters(dec_lo):
    dec_lo = np.asarray(dec_lo, np.float32)
    signs = np.where(np.arange(L) % 2 == 0, -1.0, 1.0).astype(np.float32)
    dec_hi = signs * dec_lo[::-1]
    return dec_lo[::-1].copy(), dec_hi[::-1].copy()


def band_matrix(dec_lo):
    lo_rev, hi_rev = _filters(dec_lo)
    B = np.zeros((128, BANDT), np.float32)
    for s, f in enumerate((lo_rev, hi_rev)):
        for j in range(T):
            for m in range(L):
                k = 2 * j + 1 + m
                if k < 128:
                    B[k, s * T + j] = f[m]
        for j in range(CNT_LAST):  # packed last-tile band (s-major, tight)
            for m in range(L):
                k = 2 * j + 1 + m
                B[k, BANDW + s * CNT_LAST + j] = f[m]
    return B


def taps_matrix(dec_lo):
    lo_rev, hi_rev = _filters(dec_lo)
    return np.tile(np.concatenate([lo_rev, hi_rev])[None, :], (128, 1)).copy()


def build_nc(dec_lo):
    import concourse.bacc as bacc
    import concourse.mybir as mybir
    import concourse.tile as tile

    f32 = mybir.dt.float32
    mm_dt = getattr(mybir.dt, MM_DTYPE)

    def _mmcast(ap):
        return ap if mm_dt == f32 else ap.bitcast(mm_dt)

    lo_rev, hi_rev = _filters(dec_lo)
    tap_coefs = [[float(v) for v in f] for f in (lo_rev, hi_rev)]
    CPE = C_ - F_VE                 # channels on the PE path
    CV0 = CPE                       # first VE channel
    pe_groups = [list(range(g, min(g + 4, CPE))) for g in range(0, CPE, 4)]

    nc = bacc.Bacc("TRN2", debug=False, num_devices=N_CORES)
    x = nc.dram_tensor("x", [H_, W_, C_], f32, kind="ExternalInput").ap()
    band = nc.dram_tensor("band", [128, BANDT], f32, kind="ExternalInput").ap()

    out_names = {(0, 0): "aa", (0, 1): "ad", (1, 0): "da", (1, 1): "dd"}
    outs = {
        k: nc.dram_tensor(v, [OUT, OUT, C_], f32, kind="ExternalOutput").ap()
        for k, v in out_names.items()
    }

    with tile.TileContext(nc) as tc, ExitStack() as ctx:
        xp_pool = ctx.enter_context(tc.tile_pool(name="xp", bufs=2))
        const_pool = ctx.enter_context(tc.tile_pool(name="consts", bufs=1))
        t1_pool = ctx.enter_context(tc.tile_pool(name="t1", bufs=2))
        vout_pool = ctx.enter_context(tc.tile_pool(name="vout", bufs=2))
        vtmp_pool = ctx.enter_context(tc.tile_pool(name="vtmp", bufs=1))
        stg_pool = ctx.enter_context(tc.tile_pool(name="stg", bufs=3))
        ps1_pool = ctx.enter_context(tc.tile_pool(name="ps1", bufs=3, space="PSUM"))
        ps2_pool = ctx.enter_context(tc.tile_pool(name="ps2", bufs=3, space="PSUM"))
        psv_pool = ctx.enter_context(tc.tile_pool(name="psv", bufs=2, space="PSUM"))

        bt = const_pool.tile([128, BANDT], f32)
        nc.sync.dma_start(bt[:], band[:])

        warm_ps = ps2_pool.tile([128, BANDW], f32, tag="ps2")
        for _ in range(96):  # keep HAM warm during the first input DMA
            nc.tensor.matmul(
                warm_ps[0:BANDW, :], bt[:, 0:BANDW], bt[:, 0:BANDW],
                start=True, stop=True,
            )

        def band_ap(kw, cnt):
            # contiguous [kw, 2*cnt] band slice (packed alternate for last tile)
            if cnt == T:
                return bt[0:kw, 0:BANDW]
            return bt[0:kw, BANDW:BANDW + 2 * cnt]

        # VE-path MAC balance: DVE runs fused STT MACs; GpSimd taps are an
        # ACT scale-copy into scratch plus a GpSimd tensor_tensor add
        # (Pool has no TensorScalarPtr on TRN2).
        ve_load = {"v": 0.0, "g": 0.0}
        ve_rate = {"v": 123.0, "g": 77.0}

        def ve_pick(nelem):
            key = min(ve_load, key=lambda k: ve_load[k] + nelem / ve_rate[k])
            ve_load[key] += nelem / ve_rate[key]
            return key

        for t in range(NT):
            cnt, k0, kw = _tile_params(t)
            ncols = 2 * cnt
            xp = xp_pool.tile([128, NPAD * C_], f32, tag="xp")

            # ---- load H-window (rows k0..k0+kw of padded H) with W pad ----
            p_lo = PAD - k0 if k0 < PAD else 0          # first interior partition
            hx_lo = max(0, k0 - PAD)
            p_hi = min(kw, H_ + PAD - k0)               # one past last interior
            hx_hi = hx_lo + (p_hi - p_lo)
            for tp in range(NT):                        # body, per wp-window chunk
                _, w0, ww = _tile_params(tp)
                wa = max(PAD, w0)                       # first body wp of chunk
                wb = min(PAD + W_, w0 + ww)
                load_eng = nc.sync if tp % 2 == 0 else nc.gpsimd
                load_eng.dma_start(
                    xp[p_lo:p_hi, wa * C_:wb * C_],
                    x[hx_lo:hx_hi, wa - PAD:wb - PAD, :].rearrange(
                        "h w c -> h (w c)"
                    ),
                )
            for p in range(0, p_lo):                    # top H mirror (t == 0)
                nc.sync.dma_start(
                    xp[p:p + 1, PAD * C_:(PAD + W_) * C_],
                    x[6 - p - k0:7 - p - k0].rearrange("h w c -> h (w c)"),
                )
            for p in range(p_hi, kw):                   # bottom H mirror (last t)
                hx = 2 * H_ - 1 + PAD - k0 - p
                nc.sync.dma_start(
                    xp[p:p + 1, PAD * C_:(PAD + W_) * C_],
                    x[hx:hx + 1].rearrange("h w c -> h (w c)"),
                )
            for j in range(PAD):                        # W mirror columns
                nc.gpsimd.tensor_copy(
                    xp[0:kw, j * C_:(j + 1) * C_],
                    xp[0:kw, (2 * PAD - 1 - j) * C_:(2 * PAD - j) * C_],
                )
                dst = NPAD - PAD + j
                src = PAD + W_ - 1 - j
                nc.gpsimd.tensor_copy(
                    xp[0:kw, dst * C_:(dst + 1) * C_],
                    xp[0:kw, src * C_:(src + 1) * C_],
                )

            # ---- VE stage 1: W-axis conv -> vout[p, (s', wo, cV)] ----
            if F_VE:
                vout = vout_pool.tile([128, 2 * OUT * F_VE], f32, tag="vout")
                xp_w = xp[0:kw, :].rearrange("p (w c) -> p w c", c=C_)
                vo = vout[0:kw, :].rearrange(
                    "p (s w c) -> p s w c", s=2, w=OUT
                )
                DVE_TAPS = (1, 2, 3, 5, 7)   # fused STT MACs on DVE
                for sp in range(2):
                    dst = vo[:, sp, :, :]

                    def tap_src(m):
                        return xp_w[:, 1 + m:1 + m + 2 * OUT:2, CV0:C_]

                    # chain A (ACT init + DVE STT accumulation)
                    nc.scalar.mul(dst, tap_src(0), tap_coefs[sp][0])
                    for m in DVE_TAPS:
                        nc.vector.scalar_tensor_tensor(
                            dst, tap_src(m), tap_coefs[sp][m], dst,
                            mybir.AluOpType.mult, mybir.AluOpType.add,
                        )
                    # chain B (ACT scales + GpSimd adds), combined at the end
                    vacc = vtmp_pool.tile([128, OUT * F_VE], f32, tag="vacc")
                    vtmp = vtmp_pool.tile([128, OUT * F_VE], f32, tag="vtmp")
                    vacc_r = vacc[0:kw, :].rearrange("p (w c) -> p w c", c=F_VE)
                    vtmp_r = vtmp[0:kw, :].rearrange("p (w c) -> p w c", c=F_VE)
                    nc.scalar.mul(vacc_r, tap_src(4), tap_coefs[sp][4])
                    nc.scalar.mul(vtmp_r, tap_src(6), tap_coefs[sp][6])
                    nc.gpsimd.tensor_add(vacc_r, vacc_r, vtmp_r)
                    # combine on DVE: keeps the slow GpSimd op off the
                    # S2ve critical path
                    nc.vector.tensor_add(dst, dst, vacc_r)

            # ---- PE stage 1: contract H -> T1[wp-win][w, cI*120 + (s,ho)] ----
            t1 = []
            for tp in range(NT):
                _, w0, ww = _tile_params(tp)
                t1t = t1_pool.tile([128, CPE * WSTRIDE], f32, tag=f"t1_{tp}")
                for grp in pe_groups:
                    gs = len(grp)
                    ps = ps1_pool.tile([128, 4 * WSTRIDE], f32, tag="ps1")
                    for ci, c in enumerate(grp):
                        lhsT = xp[0:kw, :].rearrange("k (w c) -> k w c", c=C_)[
                            :, w0:w0 + ww, c
                        ]
                        nc.tensor.matmul(
                            ps[0:ww, ci * WSTRIDE:ci * WSTRIDE + ncols],
                            _mmcast(lhsT),
                            _mmcast(band_ap(kw, cnt)),
                            start=True,
                            stop=True,
                        )
                    nc.scalar.copy(
                        t1t[0:ww, grp[0] * WSTRIDE:grp[0] * WSTRIDE + gs * WSTRIDE]
                        .rearrange("p (ci z) -> p ci z", ci=gs)[:, :, 0:ncols],
                        ps[0:ww, 0:gs * WSTRIDE]
                        .rearrange("p (ci z) -> p ci z", ci=gs)[:, :, 0:ncols],
                    )
                t1.append(t1t)

            # ---- stage 2 per wo-block: PE path + VE path -> shared stg ----
            for t2 in range(NT):
                cnt2, _, kw2 = _tile_params(t2)
                ncols2 = 2 * cnt2
                stg = stg_pool.tile([128, 2 * T * C_], f32, tag="stg")
                stg_r = stg[0:ncols, :].rearrange(
                    "p (s j c) -> p s j c", s=2, j=T
                )
                for grp in pe_groups:
                    gs = len(grp)
                    ps2 = ps2_pool.tile([128, 4 * WSTRIDE], f32, tag="ps2")
                    for ci, c in enumerate(grp):
                        lhsT = t1[t2][0:kw2, c * WSTRIDE:c * WSTRIDE + ncols]
                        nc.tensor.matmul(
                            ps2[0:ncols, ci * WSTRIDE:ci * WSTRIDE + ncols2],
                            _mmcast(lhsT),
                            _mmcast(band_ap(kw2, cnt2)),
                            start=True,
                            stop=True,
                        )
                    src = (
                        ps2[0:ncols, 0:gs * WSTRIDE]
                        .rearrange("p (ci z) -> p ci z", ci=gs)[:, :, 0:ncols2]
                        .rearrange("p ci (s j) -> p ci s j", s=2)
                    )
                    dst = (
                        stg_r[:, :, 0:cnt2, grp[0]:grp[0] + gs]
                        .transpose([0, 3, 1, 2])
                    )
                    nc.vector.tensor_copy(dst, src)
                # VE path stage 2: band-stationary matmul over H window
                if F_VE:
                    for sp in range(2):
                        psv = psv_pool.tile([128, T * F_VE], f32, tag="psv")
                        rhs = vo[:, sp, t2 * T:t2 * T + cnt2, :]
                        nc.tensor.matmul(
                            psv[0:ncols, 0:cnt2 * F_VE],
                            _mmcast(band_ap(kw, cnt)),
                            _mmcast(rhs),
                            start=True,
                            stop=True,
                        )
                        nc.scalar.copy(
                            stg_r[:, sp, 0:cnt2, CV0:C_],
                            psv[0:ncols, 0:cnt2 * F_VE]
                            .rearrange("p (j c) -> p j c", c=F_VE),
                        )
                for s in range(2):
                    for sp in range(2):
                        store_eng = nc.gpsimd if (s, sp) == (1, 0) else nc.scalar
                        store_eng.dma_start(
                            outs[(s, sp)][
                                t * T:t * T + cnt, t2 * T:t2 * T + cnt2, :
                            ].rearrange("h w c -> h (w c)"),
                            stg[s * cnt:(s + 1) * cnt,
                                sp * T * C_:sp * T * C_ + cnt2 * C_],
                        )

    nc.compile()
    return nc


_NC = {}


def _get_nc(dec_lo):
    key = np.asarray(dec_lo, np.float32).tobytes()
    if key not in _NC:
        _NC[key] = build_nc(dec_lo)
    return _NC[key]


def kernel(x, dec_lo):
    from concourse import bass_utils

    x = np.ascontiguousarray(np.asarray(x, np.float32))
    band = band_matrix(dec_lo)
    nc = _get_nc(dec_lo)
    in_maps = [{"x": x[i], "band": band} for i in range(N_CORES)]
    res = bass_utils.run_bass_kernel_spmd(nc, in_maps, core_ids=list(range(N_CORES)))
    names = ["aa", "ad", "da", "dd"]
    return tuple(
        np.stack([res.results[i][n] for i in range(N_CORES)], axis=0) for n in names
    )


# revision 22
# speedup vs baseline: 1.0411x; 1.0102x over previous
"""Trainium2 Bass kernel for a separable 2-D db4 DWT (pywt 'symmetric' mode).

Input  x: [8, 512, 512, 16] f32, dec_lo: [8] f32.
Output (aa, ad, da, dd): each [8, 259, 259, 16] f32.

Sharding: pure data parallel — batch i runs on core i (8 cores).

Per-core algorithm (x1: [512, 512, 16]): channels are split between two
pipelines so every engine contributes:

PE path (channels 0..15-F): both DWT stages are banded matmuls with the
*data* as the stationary operand, so the stage-1 result comes out transposed
(W on partitions) exactly as stage 2 needs it:
    stage 1:  out1[w, (s,ho)]      = sum_k Xp[k, w]   * B[k, (s,ho)]
    stage 2:  out2[(s,ho),(s',wo)] = sum_k T1[k,(s,ho)] * B[k,(s',wo)]
B is a [128, 120] banded filter matrix: B[k, s*60+j] = f_s_rev[k-(2j+1)],
covering 60 output positions of both subbands per K=128 input window.
This path is LDWEIGHTS-bound (fp32 stationary reload per matmul), so the
remaining F channels go to:

VE path (channels 16-F..15): stage 1 is the W-axis conv done as 16 fused
multiply-add sweeps (scalar_tensor_tensor) on Vector/GpSimd over the free
axis; stage 2 is the H-contraction on PE with the *band* stationary
(cheap, N~480 moving), landing (s,ho) on partitions.

Both paths interleave their stage-2 results into shared staging tiles so
output DMAs write full (wo, c) rows (3.8 KB contiguous runs).
Output tiling: position blocks of 60 (window stride 120), 5 blocks per axis.
Symmetric padding (7 each side) is materialized in SBUF: H pad via mirrored
row DMAs, W pad via small on-chip column copies.
"""

from contextlib import ExitStack

import numpy as np

B_, H_, W_, C_ = 8, 512, 512, 16
L, PAD = 8, 7
OUT = (H_ + L - 1) // 2            # 259
T = 60                             # output positions per tile
NT = (OUT + T - 1) // T            # 5
WSTRIDE = 2 * T                    # 120 (input window stride)
NPAD = H_ + 2 * PAD                # 526
N_CORES = 8
BANDW = 2 * T                      # 120 band columns (full tile)
CNT_LAST = OUT - T * (NT - 1)      # 19 positions in the last tile
BANDT = BANDW + 2 * CNT_LAST       # +38 packed columns for the last tile
F_VE = 6                           # channels on the VE path
MM_DTYPE = "float32"


def _tile_params(t):
    cnt = min(T, OUT - T * t)      # output positions in tile t
    k0 = WSTRIDE * t               # padded-axis window start
    kw = min(128, NPAD - k0)       # window size (contraction K)
    return cnt, k0, kw


def _filters(dec_lo):
    dec_lo = np.asarray(dec_lo, np.float32)
    signs = np.where(np.arange(L) % 2 == 0, -1.0, 1.0).astype(np.float32)
    dec_hi = signs * dec_lo[::-1]
    return dec_lo[::-1].copy(), dec_hi[::-1].copy()


def band_matrix(dec_lo):
    lo_rev, hi_rev = _filters(dec_lo)
    B = np.zeros((128, BANDT), np.float32)
    for s, f in enumerate((lo_rev, hi_rev)):
        for j in range(T):
            for m in range(L):
                k = 2 * j + 1 + m
                if k < 128:
                    B[k, s * T + j] = f[m]
        for j in range(CNT_LAST):  # packed last-tile band (s-major, tight)
            for m in range(L):
                k = 2 * j + 1 + m
                B[k, BANDW + s * CNT_LAST + j] = f[m]
    return B


def taps_matrix(dec_lo):
    lo_rev, hi_rev = _filters(dec_lo)
    return np.tile(np.concatenate([lo_rev, hi_rev])[None, :], (128, 1)).copy()


def build_nc(dec_lo):
    import concourse.bacc as bacc
    import concourse.mybir as mybir
    import concourse.tile as tile

    f32 = mybir.dt.float32
    mm_dt = getattr(mybir.dt, MM_DTYPE)

    def _mmcast(ap):
        return ap if mm_dt == f32 else ap.bitcast(mm_dt)

    lo_rev, hi_rev = _filters(dec_lo)
    tap_coefs = [[float(v) for v in f] for f in (lo_rev, hi_rev)]
    CPE = C_ - F_VE                 # channels on the PE path
    CV0 = CPE                       # first VE channel
    pe_groups = [list(range(g, min(g + 4, CPE))) for g in range(0, CPE, 4)]

    nc = bacc.Bacc("TRN2", debug=False, num_devices=N_CORES)
    x = nc.dram_tensor("x", [H_, W_, C_], f32, kind="ExternalInput").ap()
    band = nc.dram_tensor("band", [128, BANDT], f32, kind="ExternalInput").ap()

    out_names = {(0, 0): "aa", (0, 1): "ad", (1, 0): "da", (1, 1): "dd"}
    outs = {
        k: nc.dram_tensor(v, [OUT, OUT, C_], f32, kind="ExternalOutput").ap()
        for k, v in out_names.items()
    }

    with tile.TileContext(nc) as tc, ExitStack() as ctx:
        xp_pool = ctx.enter_context(tc.tile_pool(name="xp", bufs=2))
        const_pool = ctx.enter_context(tc.tile_pool(name="consts", bufs=1))
        t1_pool = ctx.enter_context(tc.tile_pool(name="t1", bufs=2))
        vout_pool = ctx.enter_context(tc.tile_pool(name="vout", bufs=2))
        vtmp_pool = ctx.enter_context(tc.tile_pool(name="vtmp", bufs=1))
        stg_pool = ctx.enter_context(tc.tile_pool(name="stg", bufs=3))
        ps1_pool = ctx.enter_context(tc.tile_pool(name="ps1", bufs=3, space="PSUM"))
        ps2_pool = ctx.enter_context(tc.tile_pool(name="ps2", bufs=3, space="PSUM"))
        psv_pool = ctx.enter_context(tc.tile_pool(name="psv", bufs=2, space="PSUM"))

        bt = const_pool.tile([128, BANDT], f32)
        nc.sync.dma_start(bt[:], band[:])

        warm_ps = ps2_pool.tile([128, BANDW], f32, tag="ps2")
        for _ in range(160):  # keep HAM warm during the first input DMA
            nc.tensor.matmul(
                warm_ps[0:BANDW, :], bt[:, 0:BANDW], bt[:, 0:BANDW],
                start=True, stop=True,
            )

        def band_ap(kw, cnt):
            # contiguous [kw, 2*cnt] band slice (packed alternate for last tile)
            if cnt == T:
                return bt[0:kw, 0:BANDW]
            return bt[0:kw, BANDW:BANDW + 2 * cnt]

        # VE-path MAC balance: DVE runs fused STT MACs; GpSimd taps are an
        # ACT scale-copy into scratch plus a GpSimd tensor_tensor add
        # (Pool has no TensorScalarPtr on TRN2).
        ve_load = {"v": 0.0, "g": 0.0}
        ve_rate = {"v": 123.0, "g": 77.0}

        def ve_pick(nelem):
            key = min(ve_load, key=lambda k: ve_load[k] + nelem / ve_rate[k])
            ve_load[key] += nelem / ve_rate[key]
            return key

        for t in range(NT):
            cnt, k0, kw = _tile_params(t)
            ncols = 2 * cnt
            if t:
                # bridge the inter-tile dependency stall so HAM stays warm
                warm2 = ps2_pool.tile([128, BANDW], f32, tag="ps2")
                for _ in range(8):
                    nc.tensor.matmul(
                        warm2[0:BANDW, :], bt[:, 0:BANDW], bt[:, 0:BANDW],
                        start=True, stop=True,
                    )
            xp = xp_pool.tile([128, NPAD * C_], f32, tag="xp")

            # ---- load H-window (rows k0..k0+kw of padded H) with W pad ----
            p_lo = PAD - k0 if k0 < PAD else 0          # first interior partition
            hx_lo = max(0, k0 - PAD)
            p_hi = min(kw, H_ + PAD - k0)               # one past last interior
            hx_hi = hx_lo + (p_hi - p_lo)
            for tp in range(NT):                        # body, per wp-window chunk
                _, w0, ww = _tile_params(tp)
                wa = max(PAD, w0)                       # first body wp of chunk
                wb = min(PAD + W_, w0 + ww)
                load_eng = nc.sync if tp % 2 == 0 else nc.gpsimd
                load_eng.dma_start(
                    xp[p_lo:p_hi, wa * C_:wb * C_],
                    x[hx_lo:hx_hi, wa - PAD:wb - PAD, :].rearrange(
                        "h w c -> h (w c)"
                    ),
                )
            for p in range(0, p_lo):                    # top H mirror (t == 0)
                nc.sync.dma_start(
                    xp[p:p + 1, PAD * C_:(PAD + W_) * C_],
                    x[6 - p - k0:7 - p - k0].rearrange("h w c -> h (w c)"),
                )
            for p in range(p_hi, kw):                   # bottom H mirror (last t)
                hx = 2 * H_ - 1 + PAD - k0 - p
                nc.sync.dma_start(
                    xp[p:p + 1, PAD * C_:(PAD + W_) * C_],
                    x[hx:hx + 1].rearrange("h w c -> h (w c)"),
                )
            for j in range(PAD):                        # W mirror columns
                nc.gpsimd.tensor_copy(
                    xp[0:kw, j * C_:(j + 1) * C_],
                    xp[0:kw, (2 * PAD - 1 - j) * C_:(2 * PAD - j) * C_],
                )
                dst = NPAD - PAD + j
                src = PAD + W_ - 1 - j
                nc.gpsimd.tensor_copy(
                    xp[0:kw, dst * C_:(dst + 1) * C_],
                    xp[0:kw, src * C_:(src + 1) * C_],
                )

            # ---- VE stage 1: W-axis conv -> vout[p, (s', wo, cV)] ----
            if F_VE:
                vout = vout_pool.tile([128, 2 * OUT * F_VE], f32, tag="vout")
                xp_w = xp[0:kw, :].rearrange("p (w c) -> p w c", c=C_)
                vo = vout[0:kw, :].rearrange(
                    "p (s w c) -> p s w c", s=2, w=OUT
                )
                DVE_TAPS = (1, 2, 3, 5, 7)   # fused STT MACs on DVE
                for sp in range(2):
                    dst = vo[:, sp, :, :]

                    def tap_src(m):
                        return xp_w[:, 1 + m:1 + m + 2 * OUT:2, CV0:C_]

                    # chain A (ACT init + DVE STT accumulation)
                    nc.scalar.mul(dst, tap_src(0), tap_coefs[sp][0])
                    for m in DVE_TAPS:
                        nc.vector.scalar_tensor_tensor(
                            dst, tap_src(m), tap_coefs[sp][m], dst,
                            mybir.AluOpType.mult, mybir.AluOpType.add,
                        )
                    # chain B (ACT scales + GpSimd adds), combined at the end
                    vacc = vtmp_pool.tile([128, OUT * F_VE], f32, tag="vacc")
                    vtmp = vtmp_pool.tile([128, OUT * F_VE], f32, tag="vtmp")
                    vacc_r = vacc[0:kw, :].rearrange("p (w c) -> p w c", c=F_VE)
                    vtmp_r = vtmp[0:kw, :].rearrange("p (w c) -> p w c", c=F_VE)
                    nc.scalar.mul(vacc_r, tap_src(4), tap_coefs[sp][4])
                    nc.scalar.mul(vtmp_r, tap_src(6), tap_coefs[sp][6])
                    nc.gpsimd.tensor_add(vacc_r, vacc_r, vtmp_r)
                    # combine on DVE: keeps the slow GpSimd op off the
                    # S2ve critical path
                    nc.vector.tensor_add(dst, dst, vacc_r)

            # ---- PE stage 1: contract H -> T1[wp-win][w, cI*120 + (s,ho)] ----
            t1 = []
            for tp in range(NT):
                _, w0, ww = _tile_params(tp)
                t1t = t1_pool.tile([128, CPE * WSTRIDE], f32, tag=f"t1_{tp}")
                for grp in pe_groups:
                    gs = len(grp)
                    ps = ps1_pool.tile([128, 4 * WSTRIDE], f32, tag="ps1")
                    for ci, c in enumerate(grp):
                        lhsT = xp[0:kw, :].rearrange("k (w c) -> k w c", c=C_)[
                            :, w0:w0 + ww, c
                        ]
                        nc.tensor.matmul(
                            ps[0:ww, ci * WSTRIDE:ci * WSTRIDE + ncols],
                            _mmcast(lhsT),
                            _mmcast(band_ap(kw, cnt)),
                            start=True,
                            stop=True,
                        )
                    nc.scalar.copy(
                        t1t[0:ww, grp[0] * WSTRIDE:grp[0] * WSTRIDE + gs * WSTRIDE]
                        .rearrange("p (ci z) -> p ci z", ci=gs)[:, :, 0:ncols],
                        ps[0:ww, 0:gs * WSTRIDE]
                        .rearrange("p (ci z) -> p ci z", ci=gs)[:, :, 0:ncols],
                    )
                t1.append(t1t)

            # ---- stage 2 per wo-block: PE path + VE path -> shared stg ----
            for t2 in range(NT):
                cnt2, _, kw2 = _tile_params(t2)
                ncols2 = 2 * cnt2
                stg = stg_pool.tile([128, 2 * T * C_], f32, tag="stg")
                stg_r = stg[0:ncols, :].rearrange(
                    "p (s j c) -> p s j c", s=2, j=T
                )
                for grp in pe_groups:
                    gs = len(grp)
                    ps2 = ps2_pool.tile([128, 4 * WSTRIDE], f32, tag="ps2")
                    for ci, c in enumerate(grp):
                        lhsT = t1[t2][0:kw2, c * WSTRIDE:c * WSTRIDE + ncols]
                        nc.tensor.matmul(
                            ps2[0:ncols, ci * WSTRIDE:ci * WSTRIDE + ncols2],
                            _mmcast(lhsT),
                            _mmcast(band_ap(kw2, cnt2)),
                            start=True,
                            stop=True,
                        )
                    src = (
                        ps2[0:ncols, 0:gs * WSTRIDE]
                        .rearrange("p (ci z) -> p ci z", ci=gs)[:, :, 0:ncols2]
                        .rearrange("p ci (s j) -> p ci s j", s=2)
                    )
                    dst = (
                        stg_r[:, :, 0:cnt2, grp[0]:grp[0] + gs]
                        .transpose([0, 3, 1, 2])
                    )
                    nc.vector.tensor_copy(dst, src)
                # VE path stage 2: band-stationary matmul over H window
                if F_VE:
                    for sp in range(2):
                        psv = psv_pool.tile([128, T * F_VE], f32, tag="psv")
                        rhs = vo[:, sp, t2 * T:t2 * T + cnt2, :]
                        nc.tensor.matmul(
                            psv[0:ncols, 0:cnt2 * F_VE],
                            _mmcast(band_ap(kw, cnt)),
                            _mmcast(rhs),
                            start=True,
                            stop=True,
                        )
                        nc.scalar.copy(
                            stg_r[:, sp, 0:cnt2, CV0:C_],
                            psv[0:ncols, 0:cnt2 * F_VE]
                            .rearrange("p (j c) -> p j c", c=F_VE),
                        )
                for s in range(2):
                    for sp in range(2):
                        store_eng = nc.gpsimd if (s, sp) == (1, 0) else nc.scalar
                        store_eng.dma_start(
                            outs[(s, sp)][
                                t * T:t * T + cnt, t2 * T:t2 * T + cnt2, :
                            ].rearrange("h w c -> h (w c)"),
                            stg[s * cnt:(s + 1) * cnt,
                                sp * T * C_:sp * T * C_ + cnt2 * C_],
                        )

    nc.compile()
    return nc


_NC = {}


def _get_nc(dec_lo):
    key = np.asarray(dec_lo, np.float32).tobytes()
    if key not in _NC:
        _NC[key] = build_nc(dec_lo)
    return _NC[key]


def kernel(x, dec_lo):
    from concourse import bass_utils

    x = np.ascontiguousarray(np.asarray(x, np.float32))
    band = band_matrix(dec_lo)
    nc = _get_nc(dec_lo)
    in_maps = [{"x": x[i], "band": band} for i in range(N_CORES)]
    res = bass_utils.run_bass_kernel_spmd(nc, in_maps, core_ids=list(range(N_CORES)))
    names = ["aa", "ad", "da", "dd"]
    return tuple(
        np.stack([res.results[i][n] for i in range(N_CORES)], axis=0) for n in names
    )


# revision 23
# speedup vs baseline: 1.0498x; 1.0084x over previous
"""Trainium2 Bass kernel for a separable 2-D db4 DWT (pywt 'symmetric' mode).

Input  x: [8, 512, 512, 16] f32, dec_lo: [8] f32.
Output (aa, ad, da, dd): each [8, 259, 259, 16] f32.

Sharding: pure data parallel — batch i runs on core i (8 cores).

Per-core algorithm (x1: [512, 512, 16]): channels are split between two
pipelines so every engine contributes:

PE path (channels 0..15-F): both DWT stages are banded matmuls with the
*data* as the stationary operand, so the stage-1 result comes out transposed
(W on partitions) exactly as stage 2 needs it:
    stage 1:  out1[w, (s,ho)]      = sum_k Xp[k, w]   * B[k, (s,ho)]
    stage 2:  out2[(s,ho),(s',wo)] = sum_k T1[k,(s,ho)] * B[k,(s',wo)]
B is a [128, 120] banded filter matrix: B[k, s*60+j] = f_s_rev[k-(2j+1)],
covering 60 output positions of both subbands per K=128 input window.
This path is LDWEIGHTS-bound (fp32 stationary reload per matmul), so the
remaining F channels go to:

VE path (channels 16-F..15): stage 1 is the W-axis conv done as 16 fused
multiply-add sweeps (scalar_tensor_tensor) on Vector/GpSimd over the free
axis; stage 2 is the H-contraction on PE with the *band* stationary
(cheap, N~480 moving), landing (s,ho) on partitions.

Both paths interleave their stage-2 results into shared staging tiles so
output DMAs write full (wo, c) rows (3.8 KB contiguous runs).
Output tiling: position blocks of 60 (window stride 120), 5 blocks per axis.
Symmetric padding (7 each side) is materialized in SBUF: H pad via mirrored
row DMAs, W pad via small on-chip column copies.
"""

from contextlib import ExitStack

import numpy as np

B_, H_, W_, C_ = 8, 512, 512, 16
L, PAD = 8, 7
OUT = (H_ + L - 1) // 2            # 259
T = 60                             # output positions per tile
NT = (OUT + T - 1) // T            # 5
WSTRIDE = 2 * T                    # 120 (input window stride)
NPAD = H_ + 2 * PAD                # 526
N_CORES = 8
BANDW = 2 * T                      # 120 band columns (full tile)
CNT_LAST = OUT - T * (NT - 1)      # 19 positions in the last tile
BANDT = BANDW + 2 * CNT_LAST       # +38 packed columns for the last tile
F_VE = 6                           # channels on the VE path
MM_DTYPE = "float32"


def _tile_params(t):
    cnt = min(T, OUT - T * t)      # output positions in tile t
    k0 = WSTRIDE * t               # padded-axis window start
    kw = min(128, NPAD - k0)       # window size (contraction K)
    return cnt, k0, kw


def _filters(dec_lo):
    dec_lo = np.asarray(dec_lo, np.float32)
    signs = np.where(np.arange(L) % 2 == 0, -1.0, 1.0).astype(np.float32)
    dec_hi = signs * dec_lo[::-1]
    return dec_lo[::-1].copy(), dec_hi[::-1].copy()


def band_matrix(dec_lo):
    lo_rev, hi_rev = _filters(dec_lo)
    B = np.zeros((128, BANDT), np.float32)
    for s, f in enumerate((lo_rev, hi_rev)):
        for j in range(T):
            for m in range(L):
                k = 2 * j + 1 + m
                if k < 128:
                    B[k, s * T + j] = f[m]
        for j in range(CNT_LAST):  # packed last-tile band (s-major, tight)
            for m in range(L):
                k = 2 * j + 1 + m
                B[k, BANDW + s * CNT_LAST + j] = f[m]
    return B


def taps_matrix(dec_lo):
    lo_rev, hi_rev = _filters(dec_lo)
    return np.tile(np.concatenate([lo_rev, hi_rev])[None, :], (128, 1)).copy()


def build_nc(dec_lo):
    import concourse.bacc as bacc
    import concourse.mybir as mybir
    import concourse.tile as tile

    f32 = mybir.dt.float32
    mm_dt = getattr(mybir.dt, MM_DTYPE)

    def _mmcast(ap):
        return ap if mm_dt == f32 else ap.bitcast(mm_dt)

    lo_rev, hi_rev = _filters(dec_lo)
    tap_coefs = [[float(v) for v in f] for f in (lo_rev, hi_rev)]
    CPE = C_ - F_VE                 # channels on the PE path
    CV0 = CPE                       # first VE channel
    pe_groups = [list(range(g, min(g + 4, CPE))) for g in range(0, CPE, 4)]

    nc = bacc.Bacc("TRN2", debug=False, num_devices=N_CORES)
    x = nc.dram_tensor("x", [H_, W_, C_], f32, kind="ExternalInput").ap()
    band = nc.dram_tensor("band", [128, BANDT], f32, kind="ExternalInput").ap()

    out_names = {(0, 0): "aa", (0, 1): "ad", (1, 0): "da", (1, 1): "dd"}
    outs = {
        k: nc.dram_tensor(v, [OUT, OUT, C_], f32, kind="ExternalOutput").ap()
        for k, v in out_names.items()
    }

    with tile.TileContext(nc) as tc, ExitStack() as ctx:
        xp_pool = ctx.enter_context(tc.tile_pool(name="xp", bufs=2))
        const_pool = ctx.enter_context(tc.tile_pool(name="consts", bufs=1))
        t1_pool = ctx.enter_context(tc.tile_pool(name="t1", bufs=2))
        vout_pool = ctx.enter_context(tc.tile_pool(name="vout", bufs=2))
        vtmp_pool = ctx.enter_context(tc.tile_pool(name="vtmp", bufs=1))
        stg_pool = ctx.enter_context(tc.tile_pool(name="stg", bufs=3))
        ps1_pool = ctx.enter_context(tc.tile_pool(name="ps1", bufs=3, space="PSUM"))
        ps2_pool = ctx.enter_context(tc.tile_pool(name="ps2", bufs=4, space="PSUM"))
        psv_pool = ctx.enter_context(tc.tile_pool(name="psv", bufs=1, space="PSUM"))

        bt = const_pool.tile([128, BANDT], f32)
        nc.sync.dma_start(bt[:], band[:])

        warm_ps = ps2_pool.tile([128, BANDW], f32, tag="ps2")
        for _ in range(160):  # keep HAM warm during the first input DMA
            nc.tensor.matmul(
                warm_ps[0:BANDW, :], bt[:, 0:BANDW], bt[:, 0:BANDW],
                start=True, stop=True,
            )

        def band_ap(kw, cnt):
            # contiguous [kw, 2*cnt] band slice (packed alternate for last tile)
            if cnt == T:
                return bt[0:kw, 0:BANDW]
            return bt[0:kw, BANDW:BANDW + 2 * cnt]

        # VE-path MAC balance: DVE runs fused STT MACs; GpSimd taps are an
        # ACT scale-copy into scratch plus a GpSimd tensor_tensor add
        # (Pool has no TensorScalarPtr on TRN2).
        ve_load = {"v": 0.0, "g": 0.0}
        ve_rate = {"v": 123.0, "g": 77.0}

        def ve_pick(nelem):
            key = min(ve_load, key=lambda k: ve_load[k] + nelem / ve_rate[k])
            ve_load[key] += nelem / ve_rate[key]
            return key

        for t in range(NT):
            cnt, k0, kw = _tile_params(t)
            ncols = 2 * cnt
            if t:
                # bridge the inter-tile dependency stall so HAM stays warm
                warm2 = ps2_pool.tile([128, BANDW], f32, tag="ps2")
                for _ in range(8):
                    nc.tensor.matmul(
                        warm2[0:BANDW, :], bt[:, 0:BANDW], bt[:, 0:BANDW],
                        start=True, stop=True,
                    )
            xp = xp_pool.tile([128, NPAD * C_], f32, tag="xp")

            # ---- load H-window (rows k0..k0+kw of padded H) with W pad ----
            p_lo = PAD - k0 if k0 < PAD else 0          # first interior partition
            hx_lo = max(0, k0 - PAD)
            p_hi = min(kw, H_ + PAD - k0)               # one past last interior
            hx_hi = hx_lo + (p_hi - p_lo)
            for tp in range(NT):                        # body, per wp-window chunk
                _, w0, ww = _tile_params(tp)
                wa = max(PAD, w0)                       # first body wp of chunk
                wb = min(PAD + W_, w0 + ww)
                load_eng = nc.sync if tp % 2 == 0 else nc.gpsimd
                load_eng.dma_start(
                    xp[p_lo:p_hi, wa * C_:wb * C_],
                    x[hx_lo:hx_hi, wa - PAD:wb - PAD, :].rearrange(
                        "h w c -> h (w c)"
                    ),
                )
            for p in range(0, p_lo):                    # top H mirror (t == 0)
                nc.sync.dma_start(
                    xp[p:p + 1, PAD * C_:(PAD + W_) * C_],
                    x[6 - p - k0:7 - p - k0].rearrange("h w c -> h (w c)"),
                )
            for p in range(p_hi, kw):                   # bottom H mirror (last t)
                hx = 2 * H_ - 1 + PAD - k0 - p
                nc.sync.dma_start(
                    xp[p:p + 1, PAD * C_:(PAD + W_) * C_],
                    x[hx:hx + 1].rearrange("h w c -> h (w c)"),
                )
            for j in range(PAD):                        # W mirror columns
                nc.gpsimd.tensor_copy(
                    xp[0:kw, j * C_:(j + 1) * C_],
                    xp[0:kw, (2 * PAD - 1 - j) * C_:(2 * PAD - j) * C_],
                )
                dst = NPAD - PAD + j
                src = PAD + W_ - 1 - j
                nc.gpsimd.tensor_copy(
                    xp[0:kw, dst * C_:(dst + 1) * C_],
                    xp[0:kw, src * C_:(src + 1) * C_],
                )

            # ---- VE stage 1: W-axis conv -> vout[p, (s', wo, cV)] ----
            if F_VE:
                vout = vout_pool.tile([128, 2 * OUT * F_VE], f32, tag="vout")
                xp_w = xp[0:kw, :].rearrange("p (w c) -> p w c", c=C_)
                vo = vout[0:kw, :].rearrange(
                    "p (s w c) -> p s w c", s=2, w=OUT
                )
                DVE_TAPS = (1, 2, 3, 5, 7)   # fused STT MACs on DVE
                for sp in range(2):
                    dst = vo[:, sp, :, :]

                    def tap_src(m):
                        return xp_w[:, 1 + m:1 + m + 2 * OUT:2, CV0:C_]

                    # chain A (ACT init + DVE STT accumulation)
                    nc.scalar.mul(dst, tap_src(0), tap_coefs[sp][0])
                    for m in DVE_TAPS:
                        nc.vector.scalar_tensor_tensor(
                            dst, tap_src(m), tap_coefs[sp][m], dst,
                            mybir.AluOpType.mult, mybir.AluOpType.add,
                        )
                    # chain B (ACT scales + GpSimd adds), combined at the end
                    vacc = vtmp_pool.tile([128, OUT * F_VE], f32, tag="vacc")
                    vtmp = vtmp_pool.tile([128, OUT * F_VE], f32, tag="vtmp")
                    vacc_r = vacc[0:kw, :].rearrange("p (w c) -> p w c", c=F_VE)
                    vtmp_r = vtmp[0:kw, :].rearrange("p (w c) -> p w c", c=F_VE)
                    nc.scalar.mul(vacc_r, tap_src(4), tap_coefs[sp][4])
                    nc.scalar.mul(vtmp_r, tap_src(6), tap_coefs[sp][6])
                    nc.gpsimd.tensor_add(vacc_r, vacc_r, vtmp_r)
                    # combine on DVE: keeps the slow GpSimd op off the
                    # S2ve critical path
                    nc.vector.tensor_add(dst, dst, vacc_r)

            # ---- PE stage 1: contract H -> T1[wp-win][w, cI*120 + (s,ho)] ----
            t1 = []
            for tp in range(NT):
                _, w0, ww = _tile_params(tp)
                t1t = t1_pool.tile([128, CPE * WSTRIDE], f32, tag=f"t1_{tp}")
                for grp in pe_groups:
                    gs = len(grp)
                    ps = ps1_pool.tile([128, 4 * WSTRIDE], f32, tag="ps1")
                    for ci, c in enumerate(grp):
                        lhsT = xp[0:kw, :].rearrange("k (w c) -> k w c", c=C_)[
                            :, w0:w0 + ww, c
                        ]
                        nc.tensor.matmul(
                            ps[0:ww, ci * WSTRIDE:ci * WSTRIDE + ncols],
                            _mmcast(lhsT),
                            _mmcast(band_ap(kw, cnt)),
                            start=True,
                            stop=True,
                        )
                    nc.scalar.copy(
                        t1t[0:ww, grp[0] * WSTRIDE:grp[0] * WSTRIDE + gs * WSTRIDE]
                        .rearrange("p (ci z) -> p ci z", ci=gs)[:, :, 0:ncols],
                        ps[0:ww, 0:gs * WSTRIDE]
                        .rearrange("p (ci z) -> p ci z", ci=gs)[:, :, 0:ncols],
                    )
                t1.append(t1t)

            # ---- stage 2 per wo-block: PE path + VE path -> shared stg ----
            for t2 in range(NT):
                cnt2, _, kw2 = _tile_params(t2)
                ncols2 = 2 * cnt2
                stg = stg_pool.tile([128, 2 * T * C_], f32, tag="stg")
                stg_r = stg[0:ncols, :].rearrange(
                    "p (s j c) -> p s j c", s=2, j=T
                )
                for grp in pe_groups:
                    gs = len(grp)
                    ps2 = ps2_pool.tile([128, 4 * WSTRIDE], f32, tag="ps2")
                    for ci, c in enumerate(grp):
                        lhsT = t1[t2][0:kw2, c * WSTRIDE:c * WSTRIDE + ncols]
                        nc.tensor.matmul(
                            ps2[0:ncols, ci * WSTRIDE:ci * WSTRIDE + ncols2],
                            _mmcast(lhsT),
                            _mmcast(band_ap(kw2, cnt2)),
                            start=True,
                            stop=True,
                        )
                    src = (
                        ps2[0:ncols, 0:gs * WSTRIDE]
                        .rearrange("p (ci z) -> p ci z", ci=gs)[:, :, 0:ncols2]
                        .rearrange("p ci (s j) -> p ci s j", s=2)
                    )
                    dst = (
                        stg_r[:, :, 0:cnt2, grp[0]:grp[0] + gs]
                        .transpose([0, 3, 1, 2])
                    )
                    nc.vector.tensor_copy(dst, src)
                # VE path stage 2: band-stationary matmul over H window
                if F_VE:
                    for sp in range(2):
                        psv = psv_pool.tile([128, T * F_VE], f32, tag="psv")
                        rhs = vo[:, sp, t2 * T:t2 * T + cnt2, :]
                        nc.tensor.matmul(
                            psv[0:ncols, 0:cnt2 * F_VE],
                            _mmcast(band_ap(kw, cnt)),
                            _mmcast(rhs),
                            start=True,
                            stop=True,
                        )
                        nc.scalar.copy(
                            stg_r[:, sp, 0:cnt2, CV0:C_],
                            psv[0:ncols, 0:cnt2 * F_VE]
                            .rearrange("p (j c) -> p j c", c=F_VE),
                        )
                for s in range(2):
                    for sp in range(2):
                        store_eng = nc.gpsimd if (s, sp) == (1, 0) else nc.scalar
                        store_eng.dma_start(
                            outs[(s, sp)][
                                t * T:t * T + cnt, t2 * T:t2 * T + cnt2, :
                            ].rearrange("h w c -> h (w c)"),
                            stg[s * cnt:(s + 1) * cnt,
                                sp * T * C_:sp * T * C_ + cnt2 * C_],
                        )

    nc.compile()
    return nc


_NC = {}


def _get_nc(dec_lo):
    key = np.asarray(dec_lo, np.float32).tobytes()
    if key not in _NC:
        _NC[key] = build_nc(dec_lo)
    return _NC[key]


def kernel(x, dec_lo):
    from concourse import bass_utils

    x = np.ascontiguousarray(np.asarray(x, np.float32))
    band = band_matrix(dec_lo)
    nc = _get_nc(dec_lo)
    in_maps = [{"x": x[i], "band": band} for i in range(N_CORES)]
    res = bass_utils.run_bass_kernel_spmd(nc, in_maps, core_ids=list(range(N_CORES)))
    names = ["aa", "ad", "da", "dd"]
    return tuple(
        np.stack([res.results[i][n] for i in range(N_CORES)], axis=0) for n in names
    )


# revision 25
# speedup vs baseline: 1.0569x; 1.0068x over previous
"""Trainium2 Bass kernel for a separable 2-D db4 DWT (pywt 'symmetric' mode).

Input  x: [8, 512, 512, 16] f32, dec_lo: [8] f32.
Output (aa, ad, da, dd): each [8, 259, 259, 16] f32.

Sharding: pure data parallel — batch i runs on core i (8 cores).

Per-core algorithm (x1: [512, 512, 16]): channels are split between two
pipelines so every engine contributes:

PE path (channels 0..15-F): both DWT stages are banded matmuls with the
*data* as the stationary operand, so the stage-1 result comes out transposed
(W on partitions) exactly as stage 2 needs it:
    stage 1:  out1[w, (s,ho)]      = sum_k Xp[k, w]   * B[k, (s,ho)]
    stage 2:  out2[(s,ho),(s',wo)] = sum_k T1[k,(s,ho)] * B[k,(s',wo)]
B is a [128, 120] banded filter matrix: B[k, s*60+j] = f_s_rev[k-(2j+1)],
covering 60 output positions of both subbands per K=128 input window.
This path is LDWEIGHTS-bound (fp32 stationary reload per matmul), so the
remaining F channels go to:

VE path (channels 16-F..15): stage 1 is the W-axis conv done as 16 fused
multiply-add sweeps (scalar_tensor_tensor) on Vector/GpSimd over the free
axis; stage 2 is the H-contraction on PE with the *band* stationary
(cheap, N~480 moving), landing (s,ho) on partitions.

Both paths interleave their stage-2 results into shared staging tiles so
output DMAs write full (wo, c) rows (3.8 KB contiguous runs).
Output tiling: position blocks of 60 (window stride 120), 5 blocks per axis.
Symmetric padding (7 each side) is materialized in SBUF: H pad via mirrored
row DMAs, W pad via small on-chip column copies.
"""

from contextlib import ExitStack

import numpy as np

B_, H_, W_, C_ = 8, 512, 512, 16
L, PAD = 8, 7
OUT = (H_ + L - 1) // 2            # 259
T = 60                             # output positions per tile
NT = (OUT + T - 1) // T            # 5
WSTRIDE = 2 * T                    # 120 (input window stride)
NPAD = H_ + 2 * PAD                # 526
N_CORES = 8
BANDW = 2 * T                      # 120 band columns (full tile)
CNT_LAST = OUT - T * (NT - 1)      # 19 positions in the last tile
BANDT = BANDW + 2 * CNT_LAST       # +38 packed columns for the last tile
F_VE = 6                           # channels on the VE path
MM_DTYPE = "float32"


def _tile_params(t):
    cnt = min(T, OUT - T * t)      # output positions in tile t
    k0 = WSTRIDE * t               # padded-axis window start
    kw = min(128, NPAD - k0)       # window size (contraction K)
    return cnt, k0, kw


def _filters(dec_lo):
    dec_lo = np.asarray(dec_lo, np.float32)
    signs = np.where(np.arange(L) % 2 == 0, -1.0, 1.0).astype(np.float32)
    dec_hi = signs * dec_lo[::-1]
    return dec_lo[::-1].copy(), dec_hi[::-1].copy()


def band_matrix(dec_lo):
    lo_rev, hi_rev = _filters(dec_lo)
    B = np.zeros((128, BANDT), np.float32)
    for s, f in enumerate((lo_rev, hi_rev)):
        for j in range(T):
            for m in range(L):
                k = 2 * j + 1 + m
                if k < 128:
                    B[k, s * T + j] = f[m]
        for j in range(CNT_LAST):  # packed last-tile band (s-major, tight)
            for m in range(L):
                k = 2 * j + 1 + m
                B[k, BANDW + s * CNT_LAST + j] = f[m]
    return B


def taps_matrix(dec_lo):
    lo_rev, hi_rev = _filters(dec_lo)
    return np.tile(np.concatenate([lo_rev, hi_rev])[None, :], (128, 1)).copy()


def build_nc(dec_lo):
    import concourse.bacc as bacc
    import concourse.mybir as mybir
    import concourse.tile as tile

    f32 = mybir.dt.float32
    mm_dt = getattr(mybir.dt, MM_DTYPE)

    def _mmcast(ap):
        return ap if mm_dt == f32 else ap.bitcast(mm_dt)

    lo_rev, hi_rev = _filters(dec_lo)
    tap_coefs = [[float(v) for v in f] for f in (lo_rev, hi_rev)]
    CPE = C_ - F_VE                 # channels on the PE path
    CV0 = CPE                       # first VE channel
    pe_groups = [list(range(g, min(g + 4, CPE))) for g in range(0, CPE, 4)]

    nc = bacc.Bacc("TRN2", debug=False, num_devices=N_CORES)
    x = nc.dram_tensor("x", [H_, W_, C_], f32, kind="ExternalInput").ap()
    band = nc.dram_tensor("band", [128, BANDT], f32, kind="ExternalInput").ap()

    out_names = {(0, 0): "aa", (0, 1): "ad", (1, 0): "da", (1, 1): "dd"}
    outs = {
        k: nc.dram_tensor(v, [OUT, OUT, C_], f32, kind="ExternalOutput").ap()
        for k, v in out_names.items()
    }

    with tile.TileContext(nc) as tc, ExitStack() as ctx:
        xp_pool = ctx.enter_context(tc.tile_pool(name="xp", bufs=2))
        const_pool = ctx.enter_context(tc.tile_pool(name="consts", bufs=1))
        t1_pool = ctx.enter_context(tc.tile_pool(name="t1", bufs=2))
        vout_pool = ctx.enter_context(tc.tile_pool(name="vout", bufs=2))
        vtmp_pool = ctx.enter_context(tc.tile_pool(name="vtmp", bufs=1))
        stg_pool = ctx.enter_context(tc.tile_pool(name="stg", bufs=3))
        ps1_pool = ctx.enter_context(tc.tile_pool(name="ps1", bufs=3, space="PSUM"))
        ps2_pool = ctx.enter_context(tc.tile_pool(name="ps2", bufs=4, space="PSUM"))
        psv_pool = ctx.enter_context(tc.tile_pool(name="psv", bufs=1, space="PSUM"))

        bt = const_pool.tile([128, BANDT], f32)
        nc.sync.dma_start(bt[:], band[:])

        warm_ps = ps2_pool.tile([128, BANDW], f32, tag="ps2")
        for _ in range(160):  # keep HAM warm during the first input DMA
            nc.tensor.matmul(
                warm_ps[0:BANDW, :], bt[:, 0:BANDW], bt[:, 0:BANDW],
                start=True, stop=True,
            )

        def band_ap(kw, cnt):
            # contiguous [kw, 2*cnt] band slice (packed alternate for last tile)
            if cnt == T:
                return bt[0:kw, 0:BANDW]
            return bt[0:kw, BANDW:BANDW + 2 * cnt]

        # VE-path MAC balance: DVE runs fused STT MACs; GpSimd taps are an
        # ACT scale-copy into scratch plus a GpSimd tensor_tensor add
        # (Pool has no TensorScalarPtr on TRN2).
        ve_load = {"v": 0.0, "g": 0.0}
        ve_rate = {"v": 123.0, "g": 77.0}

        def ve_pick(nelem):
            key = min(ve_load, key=lambda k: ve_load[k] + nelem / ve_rate[k])
            ve_load[key] += nelem / ve_rate[key]
            return key

        for t in range(NT):
            cnt, k0, kw = _tile_params(t)
            ncols = 2 * cnt
            if t:
                # bridge the inter-tile dependency stall so HAM stays warm
                warm2 = ps2_pool.tile([128, BANDW], f32, tag="ps2")
                for _ in range(8):
                    nc.tensor.matmul(
                        warm2[0:BANDW, :], bt[:, 0:BANDW], bt[:, 0:BANDW],
                        start=True, stop=True,
                    )
            xp = xp_pool.tile([128, NPAD * C_], f32, tag="xp")

            # ---- load H-window (rows k0..k0+kw of padded H) with W pad ----
            p_lo = PAD - k0 if k0 < PAD else 0          # first interior partition
            hx_lo = max(0, k0 - PAD)
            p_hi = min(kw, H_ + PAD - k0)               # one past last interior
            hx_hi = hx_lo + (p_hi - p_lo)
            for tp in range(NT):                        # body, per wp-window chunk
                _, w0, ww = _tile_params(tp)
                wa = max(PAD, w0)                       # first body wp of chunk
                wb = min(PAD + W_, w0 + ww)
                load_eng = nc.sync if tp % 2 == 0 else nc.gpsimd
                load_eng.dma_start(
                    xp[p_lo:p_hi, wa * C_:wb * C_],
                    x[hx_lo:hx_hi, wa - PAD:wb - PAD, :].rearrange(
                        "h w c -> h (w c)"
                    ),
                )
            for p in range(0, p_lo):                    # top H mirror (t == 0)
                nc.sync.dma_start(
                    xp[p:p + 1, PAD * C_:(PAD + W_) * C_],
                    x[6 - p - k0:7 - p - k0].rearrange("h w c -> h (w c)"),
                )
            for p in range(p_hi, kw):                   # bottom H mirror (last t)
                hx = 2 * H_ - 1 + PAD - k0 - p
                nc.sync.dma_start(
                    xp[p:p + 1, PAD * C_:(PAD + W_) * C_],
                    x[hx:hx + 1].rearrange("h w c -> h (w c)"),
                )
            for j in range(PAD):                        # W mirror columns
                nc.gpsimd.tensor_copy(
                    xp[0:kw, j * C_:(j + 1) * C_],
                    xp[0:kw, (2 * PAD - 1 - j) * C_:(2 * PAD - j) * C_],
                )
                dst = NPAD - PAD + j
                src = PAD + W_ - 1 - j
                nc.gpsimd.tensor_copy(
                    xp[0:kw, dst * C_:(dst + 1) * C_],
                    xp[0:kw, src * C_:(src + 1) * C_],
                )

            # ---- VE stage 1: W-axis conv -> vout[p, (s', wo, cV)] ----
            if F_VE:
                vout = vout_pool.tile([128, 2 * OUT * F_VE], f32, tag="vout")
                xp_w = xp[0:kw, :].rearrange("p (w c) -> p w c", c=C_)
                vo = vout[0:kw, :].rearrange(
                    "p (s w c) -> p s w c", s=2, w=OUT
                )
                DVE_TAPS = (1, 2, 3, 5, 7)   # fused STT MACs on DVE
                for sp in range(2):
                    dst = vo[:, sp, :, :]

                    def tap_src(m):
                        return xp_w[:, 1 + m:1 + m + 2 * OUT:2, CV0:C_]

                    # chain A (ACT init + DVE STT accumulation)
                    nc.scalar.mul(dst, tap_src(0), tap_coefs[sp][0])
                    for m in DVE_TAPS:
                        nc.vector.scalar_tensor_tensor(
                            dst, tap_src(m), tap_coefs[sp][m], dst,
                            mybir.AluOpType.mult, mybir.AluOpType.add,
                        )
                    # chain B (ACT scales + GpSimd adds), combined at the end
                    vacc = vtmp_pool.tile([128, OUT * F_VE], f32, tag="vacc")
                    vtmp = vtmp_pool.tile([128, OUT * F_VE], f32, tag="vtmp")
                    vacc_r = vacc[0:kw, :].rearrange("p (w c) -> p w c", c=F_VE)
                    vtmp_r = vtmp[0:kw, :].rearrange("p (w c) -> p w c", c=F_VE)
                    nc.scalar.mul(vacc_r, tap_src(4), tap_coefs[sp][4])
                    nc.scalar.mul(vtmp_r, tap_src(6), tap_coefs[sp][6])
                    nc.gpsimd.tensor_add(vacc_r, vacc_r, vtmp_r)
                    # combine on DVE: keeps the slow GpSimd op off the
                    # S2ve critical path
                    nc.vector.tensor_add(dst, dst, vacc_r)

            # ---- PE stage 1: contract H -> T1[wp-win][w, cI*120 + (s,ho)] ----
            t1 = []
            for tp in range(NT):
                _, w0, ww = _tile_params(tp)
                t1t = t1_pool.tile([128, CPE * WSTRIDE], f32, tag=f"t1_{tp}")
                for grp in pe_groups:
                    gs = len(grp)
                    ps = ps1_pool.tile([128, 4 * WSTRIDE], f32, tag="ps1")
                    for ci, c in enumerate(grp):
                        lhsT = xp[0:kw, :].rearrange("k (w c) -> k w c", c=C_)[
                            :, w0:w0 + ww, c
                        ]
                        nc.tensor.matmul(
                            ps[0:ww, ci * WSTRIDE:ci * WSTRIDE + ncols],
                            _mmcast(lhsT),
                            _mmcast(band_ap(kw, cnt)),
                            start=True,
                            stop=True,
                        )
                    nc.scalar.copy(
                        t1t[0:ww, grp[0] * WSTRIDE:grp[0] * WSTRIDE + gs * WSTRIDE]
                        .rearrange("p (ci z) -> p ci z", ci=gs)[:, :, 0:ncols],
                        ps[0:ww, 0:gs * WSTRIDE]
                        .rearrange("p (ci z) -> p ci z", ci=gs)[:, :, 0:ncols],
                    )
                t1.append(t1t)

            # ---- stage 2 per wo-block: PE path + VE path -> shared stg ----
            for t2 in range(NT):
                cnt2, _, kw2 = _tile_params(t2)
                ncols2 = 2 * cnt2
                stg = stg_pool.tile([128, 2 * T * C_], f32, tag="stg")
                stg_r = stg[0:ncols, :].rearrange(
                    "p (s j c) -> p s j c", s=2, j=T
                )
                for grp in pe_groups:
                    gs = len(grp)
                    ps2 = ps2_pool.tile([128, 4 * WSTRIDE], f32, tag="ps2")
                    for ci, c in enumerate(grp):
                        lhsT = t1[t2][0:kw2, c * WSTRIDE:c * WSTRIDE + ncols]
                        nc.tensor.matmul(
                            ps2[0:ncols, ci * WSTRIDE:ci * WSTRIDE + ncols2],
                            _mmcast(lhsT),
                            _mmcast(band_ap(kw2, cnt2)),
                            start=True,
                            stop=True,
                        )
                    src = (
                        ps2[0:ncols, 0:gs * WSTRIDE]
                        .rearrange("p (ci z) -> p ci z", ci=gs)[:, :, 0:ncols2]
                        .rearrange("p ci (s j) -> p ci s j", s=2)
                    )
                    dst = (
                        stg_r[:, :, 0:cnt2, grp[0]:grp[0] + gs]
                        .transpose([0, 3, 1, 2])
                    )
                    nc.vector.tensor_copy(dst, src)
                # VE path stage 2: band-stationary matmul over H window
                if F_VE:
                    for sp in range(2):
                        psv = psv_pool.tile([128, T * F_VE], f32, tag="psv")
                        rhs = vo[:, sp, t2 * T:t2 * T + cnt2, :]
                        nc.tensor.matmul(
                            psv[0:ncols, 0:cnt2 * F_VE],
                            _mmcast(band_ap(kw, cnt)),
                            _mmcast(rhs),
                            start=True,
                            stop=True,
                        )
                        nc.scalar.copy(
                            stg_r[:, sp, 0:cnt2, CV0:C_],
                            psv[0:ncols, 0:cnt2 * F_VE]
                            .rearrange("p (j c) -> p j c", c=F_VE),
                        )
                for s in range(2):
                    for sp in range(2):
                        store_eng = nc.gpsimd if (s, sp) == (1, 0) else nc.scalar
                        store_eng.dma_start(
                            outs[(s, sp)][
                                t * T:t * T + cnt, t2 * T:t2 * T + cnt2, :
                            ].rearrange("h w c -> h (w c)"),
                            stg[s * cnt:(s + 1) * cnt,
                                sp * T * C_:sp * T * C_ + cnt2 * C_],
                        )

    nc.compile()
    return nc


_NC = {}


def _get_nc(dec_lo):
    key = np.asarray(dec_lo, np.float32).tobytes()
    if key not in _NC:
        _NC[key] = build_nc(dec_lo)
    return _NC[key]


def kernel(x, dec_lo):
    from concourse import bass_utils

    x = np.ascontiguousarray(np.asarray(x, np.float32))
    band = band_matrix(dec_lo)
    nc = _get_nc(dec_lo)
    in_maps = [{"x": x[i], "band": band} for i in range(N_CORES)]
    res = bass_utils.run_bass_kernel_spmd(nc, in_maps, core_ids=list(range(N_CORES)))
    names = ["aa", "ad", "da", "dd"]
    return tuple(
        np.stack([res.results[i][n] for i in range(N_CORES)], axis=0) for n in names
    )
